# revision 10
# baseline (speedup 1.0000x reference)
"""Bass/Trainium2 kernel for nn_BigramLanguageModel (8 NeuronCores).

Strategy (vocab tensor-parallel lm_head):
  - The [B*T, vocab] logits + o_prob outputs (~824 MB) dominate: memory regime.
  - Each of the 8 cores replicates the tiny embed/attention/FF compute
    (~3 GFLOP total) and owns a 1/8 shard of the vocab axis for the lm_head
    matmul, the logits/softmax, and the CE-loss pieces.
  - Softmax over the full vocab needs a global row-sum of exp(logits):
    computed per-shard with a fused ACT exp+row-reduce, then AllReduce'd
    ([2048] f32, ~8 KB) across the 8 cores.
  - o_prob = exp(logit - log S) is a second streaming pass over the logits.
  - Loss: each core gathers logit[i, tgt_i] for targets in its shard via an
    indirect DMA from its logits output, reduces on device; host combines
    8 partial scalars.

Self-contained: hardcodes all shapes from the problem spec.
"""

import math
import numpy as np

import concourse.bass as bass
from concourse import bacc, mybir
from concourse.tile import TileContext
from concourse.masks import make_identity
from concourse.bass_utils import run_bass_kernel_spmd

# Problem shapes (hardcoded per contract)
V, E, BL, HID, NH = 50257, 256, 512, 1024, 8
HD = E // NH              # 32
B, T = 4, 512
R = B * T                 # 2048 rows
NCORES = 8
VS = math.ceil(V / NCORES)       # 6283 per-core vocab shard (last core padded)
VT = 512                         # vocab tile width
NVT = math.ceil(VS / VT)         # 13 tiles (last = 139)
RB = R // 128                    # 16 row blocks
ECH = E // 128                   # 2 embed chunks
HCH = HID // 128                 # 8 hidden chunks
TGT_SLOTS = 2048                 # target-gather slots (covers any distribution)
NEG_BIG = -1.0e30

_f32 = mybir.dt.float32
_f32r = mybir.dt.float32r
_i32 = mybir.dt.int32

_CACHED_NC = None


def _vt_width(vt):
    return VT if vt < NVT - 1 else VS - VT * (NVT - 1)


def _build_program():
    nc = bacc.Bacc(num_devices=NCORES)

    idx_d = nc.declare_dram_parameter("idx_flat", [R], _i32, isOutput=False)
    tok_d = nc.declare_dram_parameter("tok_emb", [V, E], _f32, isOutput=False)
    pos_d = nc.declare_dram_parameter("pos_emb", [BL, E], _f32, isOutput=False)
    wq_d = nc.declare_dram_parameter("Wq", [NH, E, HD], _f32, isOutput=False)
    wk_d = nc.declare_dram_parameter("Wk", [NH, E, HD], _f32, isOutput=False)
    wv_d = nc.declare_dram_parameter("Wv", [NH, E, HD], _f32, isOutput=False)
    w1_d = nc.declare_dram_parameter("W1", [E, HID], _f32, isOutput=False)
    b1_d = nc.declare_dram_parameter("b1", [HID], _f32, isOutput=False)
    w2_d = nc.declare_dram_parameter("w2s", [HID, VS], _f32, isOutput=False)
    b2_d = nc.declare_dram_parameter("b2s", [VS], _f32, isOutput=False)
    tgt_d = nc.declare_dram_parameter("tgt_flat", [TGT_SLOTS], _i32, isOutput=False)
    tgw_d = nc.declare_dram_parameter("tgt_w", [TGT_SLOTS], _f32, isOutput=False)

    logit_o = nc.declare_dram_parameter("logits_s", [R, VS], _f32, isOutput=True)
    oprob_o = nc.declare_dram_parameter("oprob_s", [R, VS], _f32, isOutput=True)
    loss_o = nc.declare_dram_parameter("loss_acc", [1, 2], _f32, isOutput=True)

    with TileContext(nc) as tc:
        with tc.tile_pool(name="const", bufs=1) as const:
            hT = const.tile([128, HCH, R], _f32r)          # 64 KB/part
            S_parts = const.tile([128, RB, 16], _f32)

            # ---------------- Phase A: embed + attention + FF ----------------
            with tc.tile_pool(name="attn", bufs=1) as ap_, \
                 tc.tile_pool(name="wrkA", bufs=3) as wrkA, \
                 tc.tile_pool(name="psA", bufs=4, space="PSUM") as psA, \
                 tc.tile_pool(name="psO", bufs=2, space="PSUM") as psO:

                idx_sb = ap_.tile([128, RB], _i32)
                nc.sync.dma_start(out=idx_sb, in_=idx_d[:].rearrange("(c p) -> p c", p=128))

                # gather x = tok_emb[idx]  (row per partition), add pos_emb
                xg = ap_.tile([128, RB, E], _f32)
                for c in range(RB):
                    nc.gpsimd.indirect_dma_start(
                        out=xg[:, c, :], out_offset=None,
                        in_=tok_d[:, :],
                        in_offset=bass.IndirectOffsetOnAxis(ap=idx_sb[:, c:c + 1], axis=0),
                    )
                pos_sb = ap_.tile([128, 4, E], _f32)
                nc.sync.dma_start(out=pos_sb, in_=pos_d[:, :].rearrange("(c p) e -> p c e", p=128))
                for c in range(RB):
                    nc.vector.tensor_add(out=xg[:, c, :], in0=xg[:, c, :], in1=pos_sb[:, c % 4, :])

                # transpose to xT [e, tokens]
                ident = ap_.tile([128, 128], _f32)
                make_identity(nc, ident)
                xT = ap_.tile([128, ECH, R], _f32r)
                for c in range(RB):
                    for e in range(ECH):
                        pt = psA.tile([128, 128], _f32, tag="ps")
                        nc.tensor.transpose(out=pt, in_=xg[:, c, e * 128:(e + 1) * 128], identity=ident)
                        nc.vector.tensor_copy(out=xT[:, e, c * 128:(c + 1) * 128], in_=pt)

                # attention weights
                wq_sb = ap_.tile([128, ECH, NH, HD], _f32r)
                wk_sb = ap_.tile([128, ECH, NH, HD], _f32r)
                wv_sb = ap_.tile([128, ECH, NH, HD], _f32r)
                for c in range(ECH):
                    for w_d, w_sb in ((wq_d, wq_sb), (wk_d, wk_sb), (wv_d, wv_sb)):
                        nc.sync.dma_start(
                            out=w_sb[:, c],
                            in_=w_d[:, c * 128:(c + 1) * 128, :].rearrange("n p h -> p n h").bitcast(_f32r))

                # v = x @ Wv for all heads:  [tok, (n h)]
                v_all = ap_.tile([128, RB, NH * HD], _f32r)
                for tb in range(RB):
                    pv = psA.tile([128, NH * HD], _f32, tag="ps")
                    for c in range(ECH):
                        nc.tensor.matmul(
                            pv,
                            xT[:, c, tb * 128:(tb + 1) * 128],
                            wv_sb[:, c].rearrange("p n h -> p (n h)"),
                            start=(c == 0), stop=(c == ECH - 1),
                        )
                    nc.vector.tensor_copy(out=v_all[:, tb, :], in_=pv)

                attnT = ap_.tile([128, ECH, R], _f32r)
                inv_sqrt_c = 1.0 / math.sqrt(E)   # NOTE: reference scales by EMBED size
                for b in range(B):
                    for cc in range(ECH):
                        for nn in range(4):
                            n = cc * 4 + nn
                            pq = psA.tile([32, T], _f32, tag="ps")
                            pk = psA.tile([32, T], _f32, tag="ps")
                            for c in range(ECH):
                                nc.tensor.matmul(pq, wq_sb[:, c, n, :],
                                                 xT[:, c, b * T:(b + 1) * T],
                                                 start=(c == 0), stop=(c == ECH - 1))
                            for c in range(ECH):
                                nc.tensor.matmul(pk, wk_sb[:, c, n, :],
                                                 xT[:, c, b * T:(b + 1) * T],
                                                 start=(c == 0), stop=(c == ECH - 1))
                            qT = wrkA.tile([32, T], _f32r, tag="qT")
                            kT = wrkA.tile([32, T], _f32r, tag="kT")
                            # fold the 1/sqrt(E) score scale into q
                            nc.scalar.activation(out=qT, in_=pq,
                                                 func=mybir.ActivationFunctionType.Copy,
                                                 scale=inv_sqrt_c)
                            nc.vector.tensor_copy(out=kT, in_=pk)

                            e_nb = wrkA.tile([128, 4, T], _f32r, tag="enb")
                            dcol = wrkA.tile([128, 4], _f32, tag="dcol")
                            for sb in range(4):
                                psc = psA.tile([128, T], _f32, tag="ps")
                                # scoresT[s, t] = k[s] . q[t]
                                nc.tensor.matmul(psc, kT[:, sb * 128:(sb + 1) * 128],
                                                 qT, start=True, stop=True)
                                # exp then causal-zero (t >= s kept); scores are tiny so
                                # max-subtraction is unnecessary in fp32
                                nc.scalar.activation(out=e_nb[:, sb, :], in_=psc,
                                                     func=mybir.ActivationFunctionType.Exp)
                                nc.gpsimd.affine_select(
                                    out=e_nb[:, sb, :], in_=e_nb[:, sb, :],
                                    compare_op=mybir.AluOpType.is_ge, fill=0.0,
                                    base=-(sb * 128), pattern=[[1, T]], channel_multiplier=-1,
                                )
                                # softmax (over the query axis t) denominator, per key s
                                nc.vector.tensor_reduce(out=dcol[:, sb:sb + 1], in_=e_nb[:, sb, :],
                                                        axis=mybir.AxisListType.X,
                                                        op=mybir.AluOpType.add)
                            drec = wrkA.tile([128, 4], _f32, tag="drec")
                            nc.vector.reciprocal(out=drec, in_=dcol)
                            vprime = wrkA.tile([128, 4, HD], _f32r, tag="vp")
                            for sb in range(4):
                                nc.vector.tensor_scalar(
                                    out=vprime[:, sb, :],
                                    in0=v_all[:, b * 4 + sb, n * HD:(n + 1) * HD],
                                    scalar1=drec[:, sb:sb + 1], scalar2=None,
                                    op0=mybir.AluOpType.mult,
                                )
                            # outT[h, t] = sum_s v'(s,h) e(s,t); per-head PSUM at
                            # base partition 0, then SBUF->SBUF DMA moves the
                            # [32, T] block to its head's partition slice
                            po = psO.tile([32, T], _f32, tag="pso")
                            for sb in range(4):
                                nc.tensor.matmul(
                                    po,
                                    vprime[:, sb, :],
                                    e_nb[:, sb, :],
                                    start=(sb == 0), stop=(sb == 3),
                                )
                            o_sb = wrkA.tile([32, T], _f32r, tag="osb")
                            nc.vector.tensor_copy(out=o_sb, in_=po)
                            nc.sync.dma_start(
                                out=attnT[nn * 32:(nn + 1) * 32, cc, b * T:(b + 1) * T],
                                in_=o_sb)

                # FF: hT = relu(W1^T @ attnT + b1)
                w1_sb = ap_.tile([128, ECH, HID], _f32r)
                nc.sync.dma_start(out=w1_sb, in_=w1_d[:, :].rearrange("(c p) h -> p c h", p=128).bitcast(_f32r))
                b1_sb = ap_.tile([128, HCH], _f32)
                nc.sync.dma_start(out=b1_sb, in_=b1_d[:].rearrange("(c p) -> p c", p=128))
                for hb in range(HCH):
                    for t4 in range(R // T):
                        ph = psO.tile([128, T], _f32, tag="pso")
                        for c in range(ECH):
                            nc.tensor.matmul(ph, w1_sb[:, c, hb * 128:(hb + 1) * 128],
                                             attnT[:, c, t4 * T:(t4 + 1) * T],
                                             start=(c == 0), stop=(c == ECH - 1))
                        nc.scalar.activation(out=hT[:, hb, t4 * T:(t4 + 1) * T], in_=ph,
                                             func=mybir.ActivationFunctionType.Relu,
                                             bias=b1_sb[:, hb:hb + 1], scale=1.0)

            # ---------------- Phase B: lm_head shard + local row-sums ----------------
            with tc.tile_pool(name="w2p", bufs=2) as w2p, \
                 tc.tile_pool(name="lb", bufs=4) as lb, \
                 tc.tile_pool(name="psB", bufs=4, space="PSUM") as psB:
                for vt in range(NVT):
                    w = _vt_width(vt)
                    w2t = w2p.tile([128, HCH, VT], _f32r, tag="w2t")
                    nc.sync.dma_start(out=w2t[:, :, :w],
                                      in_=w2_d[:, vt * VT:vt * VT + w].rearrange("(c p) v -> p c v", p=128).bitcast(_f32r))
                    b2t = w2p.tile([128, VT], _f32, tag="b2t")
                    b2_slice = b2_d[vt * VT:vt * VT + w]
                    b2_bcast = bass.AP(tensor=b2_slice.tensor, offset=b2_slice.offset,
                                       ap=[[0, 128]] + list(b2_slice.ap))
                    nc.sync.dma_start(out=b2t[:, :w], in_=b2_bcast)
                    wmm = (w + 3) // 4 * 4   # f32r matmul needs an aligned free dim
                    for rb in range(RB):
                        pl = psB.tile([128, VT], _f32, tag="pl")
                        for k in range(HCH):
                            nc.tensor.matmul(pl[:, :wmm], hT[:, k, rb * 128:(rb + 1) * 128],
                                             w2t[:, k, :wmm],
                                             start=(k == 0), stop=(k == HCH - 1))
                        lt = lb.tile([128, VT], _f32, tag="lt")
                        nc.vector.tensor_add(out=lt[:, :w], in0=pl[:, :w], in1=b2t[:, :w])
                        nc.sync.dma_start(out=logit_o[rb * 128:(rb + 1) * 128, vt * VT:vt * VT + w],
                                          in_=lt[:, :w])
                        et = lb.tile([128, VT], _f32, tag="et")
                        nc.scalar.activation(out=et[:, :w], in_=lt[:, :w],
                                             func=mybir.ActivationFunctionType.Exp,
                                             accum_out=S_parts[:, rb, vt:vt + 1])

            # ---------------- Phase C: AllReduce row-sums; loss pieces ----------------
            S_loc = const.tile([128, RB], _f32)
            nc.vector.tensor_reduce(out=S_loc, in_=S_parts[:, :, :NVT],
                                    axis=mybir.AxisListType.X, op=mybir.AluOpType.add)
            S_glob = const.tile([128, RB], _f32)
            with tc.tile_pool(name="dramp", bufs=1, space="DRAM") as dram:
                cc_in = dram.tile([128, RB], _f32)
                cc_out = dram.tile([128, RB], _f32)
                nc.sync.dma_start(out=cc_in[:], in_=S_loc)
                nc.gpsimd.collective_compute(
                    "AllReduce", mybir.AluOpType.add,
                    replica_groups=[list(range(NCORES))],
                    ins=[cc_in.opt()], outs=[cc_out.opt()],
                )
                nc.sync.dma_start(out=S_glob, in_=cc_out[:])

            logS = const.tile([128, RB], _f32)
            nc.scalar.activation(out=logS, in_=S_glob, func=mybir.ActivationFunctionType.Ln)
            neglogS = const.tile([128, RB], _f32)
            nc.vector.tensor_scalar_mul(out=neglogS, in0=logS, scalar1=-1.0)

            # loss pieces: sum(logS) over all rows, and sum of target logits
            # owned by this shard (indices/pads prepared on host from targets)
            red1 = const.tile([128, 1], _f32)
            nc.vector.tensor_reduce(out=red1, in_=logS, axis=mybir.AxisListType.X,
                                    op=mybir.AluOpType.add)
            red2 = const.tile([1, 1], _f32)
            nc.gpsimd.tensor_reduce(out=red2, in_=red1, axis=mybir.AxisListType.C,
                                    op=mybir.AluOpType.add)
            nc.sync.dma_start(out=loss_o[0:1, 1:2], in_=red2)

            ngat = TGT_SLOTS // 128
            tgt_sb = const.tile([128, ngat], _i32)
            nc.sync.dma_start(out=tgt_sb, in_=tgt_d[:].rearrange("(g p) -> p g", p=128))
            tgw_sb = const.tile([128, ngat], _f32)
            nc.sync.dma_start(out=tgw_sb, in_=tgw_d[:].rearrange("(g p) -> p g", p=128))
            lo_ap = logit_o[:, :]
            flat_logits = bass.AP(tensor=lo_ap.tensor, offset=0, ap=[[1, R * VS], [1, 1]])
            gat = const.tile([128, ngat], _f32)
            for g in range(ngat):
                nc.gpsimd.indirect_dma_start(
                    out=gat[:, g:g + 1], out_offset=None,
                    in_=flat_logits,
                    in_offset=bass.IndirectOffsetOnAxis(ap=tgt_sb[:, g:g + 1], axis=0),
                )
            gatw = const.tile([128, ngat], _f32)
            nc.vector.tensor_mul(out=gatw, in0=gat, in1=tgw_sb)
            red3 = const.tile([128, 1], _f32)
            nc.vector.tensor_reduce(out=red3, in_=gatw, axis=mybir.AxisListType.X,
                                    op=mybir.AluOpType.add)
            red4 = const.tile([1, 1], _f32)
            nc.gpsimd.tensor_reduce(out=red4, in_=red3, axis=mybir.AxisListType.C,
                                    op=mybir.AluOpType.add)
            nc.sync.dma_start(out=loss_o[0:1, 0:1], in_=red4)

            # ---------------- Phase D: o_prob = exp(logit - logS) ----------------
            with tc.tile_pool(name="od", bufs=4) as od:
                for vt in range(NVT):
                    w = _vt_width(vt)
                    for rb in range(RB):
                        lt2 = od.tile([128, VT], _f32, tag="lt2")
                        nc.sync.dma_start(out=lt2[:, :w],
                                          in_=logit_o[rb * 128:(rb + 1) * 128, vt * VT:vt * VT + w])
                        ot = od.tile([128, VT], _f32, tag="ot")
                        nc.scalar.activation(out=ot[:, :w], in_=lt2[:, :w],
                                             func=mybir.ActivationFunctionType.Exp,
                                             bias=neglogS[:, rb:rb + 1], scale=1.0)
                        nc.sync.dma_start(out=oprob_o[rb * 128:(rb + 1) * 128, vt * VT:vt * VT + w],
                                          in_=ot[:, :w])

    nc.compile()
    return nc


def _get_program():
    global _CACHED_NC
    if _CACHED_NC is None:
        _CACHED_NC = _build_program()
    return _CACHED_NC


def _make_in_maps(ins):
    idx = np.ascontiguousarray(np.asarray(ins["idx"]).reshape(-1).astype(np.int32))
    targets = np.asarray(ins["targets"]).reshape(-1).astype(np.int64)
    tok_emb = np.ascontiguousarray(np.asarray(ins["tok_emb"], dtype=np.float32))
    pos_emb = np.ascontiguousarray(np.asarray(ins["pos_emb"], dtype=np.float32))
    wq = np.ascontiguousarray(np.asarray(ins["Wq"], dtype=np.float32))
    wk = np.ascontiguousarray(np.asarray(ins["Wk"], dtype=np.float32))
    wv = np.ascontiguousarray(np.asarray(ins["Wv"], dtype=np.float32))
    w1 = np.ascontiguousarray(np.asarray(ins["W1"], dtype=np.float32))
    b1 = np.ascontiguousarray(np.asarray(ins["b1"], dtype=np.float32))
    w2 = np.asarray(ins["W2"], dtype=np.float32)
    b2 = np.asarray(ins["b2"], dtype=np.float32)

    in_maps = []
    for v in range(NCORES):
        lo = v * VS
        hi = min(lo + VS, V)
        wdt = hi - lo
        w2s = np.zeros((HID, VS), dtype=np.float32)
        w2s[:, :wdt] = w2[:, lo:hi]
        b2s = np.full((VS,), NEG_BIG, dtype=np.float32)
        b2s[:wdt] = b2[lo:hi]
        # flat indices (row*VS + local col) of targets owned by this shard
        rows = np.nonzero((targets >= lo) & (targets < hi))[0]
        tgt_flat = np.zeros((TGT_SLOTS,), dtype=np.int32)
        tgt_w = np.zeros((TGT_SLOTS,), dtype=np.float32)
        n = len(rows)
        tgt_flat[:n] = (rows * VS + (targets[rows] - lo)).astype(np.int32)
        tgt_w[:n] = 1.0
        in_maps.append({
            "idx_flat": idx,
            "tok_emb": tok_emb,
            "pos_emb": pos_emb,
            "Wq": wq, "Wk": wk, "Wv": wv,
            "W1": w1, "b1": b1,
            "w2s": np.ascontiguousarray(w2s),
            "b2s": b2s,
            "tgt_flat": tgt_flat,
            "tgt_w": tgt_w,
        })
    return in_maps


def run_sharded(ins, trace=False):
    """Build+run the SPMD program; returns (logits, loss, o_prob, results)."""
    nc = _get_program()
    in_maps = _make_in_maps(ins)
    res = run_bass_kernel_spmd(nc, in_maps, list(range(NCORES)), trace=trace)
    logits = np.concatenate(
        [res.results[v]["logits_s"] for v in range(NCORES)], axis=1)[:, :V]
    o_prob = np.concatenate(
        [res.results[v]["oprob_s"] for v in range(NCORES)], axis=1)[:, :V]
    acc = np.stack([res.results[v]["loss_acc"] for v in range(NCORES)])  # [8,1,2]
    sum_logS = float(acc[0, 0, 1])
    sum_tgt = float(acc[:, 0, 0].sum())
    loss = np.float32((sum_logS - sum_tgt) / R)
    return logits, loss, o_prob, res


def kernel(**inputs):
    logits, loss, o_prob, _ = run_sharded(inputs, trace=False)
    return logits, loss, o_prob


# revision 14
# speedup vs baseline: 1.0160x; 1.0160x over previous
"""Bass/Trainium2 kernel for nn_BigramLanguageModel (8 NeuronCores).

Strategy (vocab tensor-parallel lm_head):
  - The [B*T, vocab] logits + o_prob outputs (~824 MB) dominate: memory regime.
  - Each of the 8 cores replicates the tiny embed/attention/FF compute
    (~3 GFLOP) and owns a 1/8 shard of the vocab axis for the lm_head
    matmul, the logits/softmax, and the CE-loss pieces.
  - W2's shard lives resident in SBUF as bf16 (100.5 KB/partition), so the
    lm_head runs row-block-outer in a single fused streaming pass:
    matmul -> logits store (straight from PSUM) -> fused exp+row-sum, with
    the exp tiles kept in SBUF (bf16). Each row block's softmax denominator
    is AllReduce'd ([128] f32) across the 8 cores, pipelined behind the next
    row block's matmuls, then o_prob = e * (1/S) streams out.
  - Loss: each core gathers logit[i, tgt_i] for targets in its shard via an
    indirect DMA from its logits output and reduces on device; the host
    combines 8 partial scalars.

Self-contained: hardcodes all shapes from the problem spec.
"""

import math
import numpy as np
import ml_dtypes

import concourse.bass as bass
from concourse import bacc, mybir
from concourse.tile import TileContext
from concourse.masks import make_identity
from concourse.bass_utils import run_bass_kernel_spmd

# Problem shapes (hardcoded per contract)
V, E, BL, HID, NH = 50257, 256, 512, 1024, 8
HD = E // NH              # 32
B, T = 4, 512
R = B * T                 # 2048 rows
NCORES = 8
VS = math.ceil(V / NCORES)       # 6283 per-core vocab shard (last core padded)
VT = 512                         # vocab tile width
NVT = math.ceil(VS / VT)         # 13 tiles (last = 139)
RB = R // 128                    # 16 row blocks
ECH = E // 128                   # 2 embed chunks
HCH = HID // 128                 # 8 hidden chunks
TGT_SLOTS = 2048                 # target-gather slots (covers any distribution)

_f32 = mybir.dt.float32
_f32r = mybir.dt.float32r
_bf16 = mybir.dt.bfloat16
_i32 = mybir.dt.int32

_CACHE = {}


def _vt_width(vt):
    return VT if vt < NVT - 1 else VS - VT * (NVT - 1)


def _build_program(b2_zero):
    nc = bacc.Bacc(num_devices=NCORES)

    idx_d = nc.declare_dram_parameter("idx_flat", [R], _i32, isOutput=False)
    tok_d = nc.declare_dram_parameter("tok_emb", [V, E], _f32, isOutput=False)
    pos_d = nc.declare_dram_parameter("pos_emb", [BL, E], _f32, isOutput=False)
    wq_d = nc.declare_dram_parameter("Wq", [NH, E, HD], _f32, isOutput=False)
    wk_d = nc.declare_dram_parameter("Wk", [NH, E, HD], _f32, isOutput=False)
    wv_d = nc.declare_dram_parameter("Wv", [NH, E, HD], _f32, isOutput=False)
    w1_d = nc.declare_dram_parameter("W1", [E, HID], _f32, isOutput=False)
    b1_d = nc.declare_dram_parameter("b1", [HID], _f32, isOutput=False)
    w2_d = nc.declare_dram_parameter("w2s", [HID, VS], _bf16, isOutput=False)
    b2_d = nc.declare_dram_parameter("b2s", [VS], _f32, isOutput=False)
    tgt_d = nc.declare_dram_parameter("tgt_flat", [TGT_SLOTS], _i32, isOutput=False)
    tgw_d = nc.declare_dram_parameter("tgt_w", [TGT_SLOTS], _f32, isOutput=False)

    logit_o = nc.declare_dram_parameter("logits_s", [R, VS], _f32, isOutput=True)
    oprob_o = nc.declare_dram_parameter("oprob_s", [R, VS], _f32, isOutput=True)
    loss_o = nc.declare_dram_parameter("loss_acc", [1, 2], _f32, isOutput=True)

    inv_sqrt_c = 1.0 / math.sqrt(E)   # NOTE: reference scales scores by EMBED size

    with TileContext(nc) as tc:
        with tc.tile_pool(name="const", bufs=1) as const:
            hT = const.tile([128, HCH, R], _bf16)           # 32 KB/part
            S_parts = const.tile([128, RB, 16], _f32)
            S_glob = const.tile([128, RB], _f32)
            logS = const.tile([128, RB], _f32)

            # ---------------- Phase A: embed + attention + FF ----------------
            with tc.tile_pool(name="attn", bufs=1) as ap_, \
                 tc.tile_pool(name="wrkA", bufs=2) as wrkA, \
                 tc.tile_pool(name="psA", bufs=4, space="PSUM") as psA, \
                 tc.tile_pool(name="psO", bufs=2, space="PSUM") as psO:

                idx_sb = ap_.tile([128, RB], _i32)
                nc.sync.dma_start(out=idx_sb, in_=idx_d[:].rearrange("(c p) -> p c", p=128))

                # gather x = tok_emb[idx] (row per partition), add pos_emb
                xg = ap_.tile([128, RB, E], _f32)
                for c in range(RB):
                    nc.gpsimd.indirect_dma_start(
                        out=xg[:, c, :], out_offset=None,
                        in_=tok_d[:, :],
                        in_offset=bass.IndirectOffsetOnAxis(ap=idx_sb[:, c:c + 1], axis=0),
                    )
                pos_sb = ap_.tile([128, 4, E], _f32)
                nc.sync.dma_start(out=pos_sb, in_=pos_d[:, :].rearrange("(c p) e -> p c e", p=128))
                for c in range(RB):
                    nc.vector.tensor_add(out=xg[:, c, :], in0=xg[:, c, :], in1=pos_sb[:, c % 4, :])

                # transpose to xT [e, tokens]
                ident = ap_.tile([128, 128], _f32)
                make_identity(nc, ident)
                xT = ap_.tile([128, ECH, R], _f32r)
                for c in range(RB):
                    for e in range(ECH):
                        pt = psA.tile([128, 128], _f32, tag="ps")
                        nc.tensor.transpose(out=pt, in_=xg[:, c, e * 128:(e + 1) * 128], identity=ident)
                        nc.vector.tensor_copy(out=xT[:, e, c * 128:(c + 1) * 128], in_=pt)

                # attention weights (per-head layout [p, chunk, n, h])
                wq_sb = ap_.tile([128, ECH, NH, HD], _f32r)
                wk_sb = ap_.tile([128, ECH, NH, HD], _f32r)
                wv_sb = ap_.tile([128, ECH, NH, HD], _f32r)
                for c in range(ECH):
                    for w_d, w_sb in ((wq_d, wq_sb), (wk_d, wk_sb), (wv_d, wv_sb)):
                        nc.sync.dma_start(
                            out=w_sb[:, c],
                            in_=w_d[:, c * 128:(c + 1) * 128, :].rearrange("n p h -> p n h").bitcast(_f32r))

                # v = x @ Wv for all heads:  [tok, (n h)]
                v_all = ap_.tile([128, RB, NH * HD], _f32r)
                for tb in range(RB):
                    pv = psA.tile([128, NH * HD], _f32, tag="ps")
                    for c in range(ECH):
                        nc.tensor.matmul(
                            pv,
                            xT[:, c, tb * 128:(tb + 1) * 128],
                            wv_sb[:, c].rearrange("p n h -> p (n h)"),
                            start=(c == 0), stop=(c == ECH - 1),
                        )
                    nc.vector.tensor_copy(out=v_all[:, tb, :], in_=pv)

                attnT = ap_.tile([128, ECH, R], _f32r)
                for b in range(B):
                    for cc in range(ECH):
                        # q/k for this chunk's 4 heads in one [128, T] matmul pair:
                        # head nn lives at partitions nn*32..nn*32+32
                        pq = psA.tile([128, T], _f32, tag="ps")
                        pk = psA.tile([128, T], _f32, tag="ps")
                        for c in range(ECH):
                            nc.tensor.matmul(
                                pq, wq_sb[:, c, cc * 4:(cc + 1) * 4].rearrange("p n h -> p (n h)"),
                                xT[:, c, b * T:(b + 1) * T],
                                start=(c == 0), stop=(c == ECH - 1))
                        for c in range(ECH):
                            nc.tensor.matmul(
                                pk, wk_sb[:, c, cc * 4:(cc + 1) * 4].rearrange("p n h -> p (n h)"),
                                xT[:, c, b * T:(b + 1) * T],
                                start=(c == 0), stop=(c == ECH - 1))
                        q4 = wrkA.tile([128, T], _f32r, tag="q4")
                        k4 = wrkA.tile([128, T], _f32r, tag="k4")
                        nc.vector.tensor_copy(out=q4, in_=pq)
                        nc.vector.tensor_copy(out=k4, in_=pk)

                        for nn in range(4):
                            n = cc * 4 + nn
                            tp = {"tile_position": (nn * 32, 0)} if nn * 32 > 64 else {}
                            e_nb = wrkA.tile([128, 4, T], _f32r, tag="enb")
                            dcol = wrkA.tile([128, 4], _f32, tag="dcol")
                            for sb in range(4):
                                psc = psA.tile([128, T], _f32, tag="ps")
                                # scoresT[s, t] = k[s] . q[t]  (unscaled)
                                nc.tensor.matmul(psc,
                                                 k4[nn * 32:(nn + 1) * 32, sb * 128:(sb + 1) * 128],
                                                 q4[nn * 32:(nn + 1) * 32, :],
                                                 start=True, stop=True, **tp)
                                # exp(score / sqrt(E)); scores are tiny so no max-sub
                                nc.scalar.activation(out=e_nb[:, sb, :], in_=psc,
                                                     func=mybir.ActivationFunctionType.Exp,
                                                     scale=inv_sqrt_c)
                                # causal: keep t >= s, else 0
                                nc.gpsimd.affine_select(
                                    out=e_nb[:, sb, :], in_=e_nb[:, sb, :],
                                    compare_op=mybir.AluOpType.is_ge, fill=0.0,
                                    base=-(sb * 128), pattern=[[1, T]], channel_multiplier=-1,
                                )
                                # softmax-over-query-axis denominator, per key s
                                nc.vector.tensor_reduce(out=dcol[:, sb:sb + 1], in_=e_nb[:, sb, :],
                                                        axis=mybir.AxisListType.X,
                                                        op=mybir.AluOpType.add)
                            drec = wrkA.tile([128, 4], _f32, tag="drec")
                            nc.vector.reciprocal(out=drec, in_=dcol)
                            vprime = wrkA.tile([128, 4, HD], _f32r, tag="vp")
                            for sb in range(4):
                                nc.vector.tensor_scalar(
                                    out=vprime[:, sb, :],
                                    in0=v_all[:, b * 4 + sb, n * HD:(n + 1) * HD],
                                    scalar1=drec[:, sb:sb + 1], scalar2=None,
                                    op0=mybir.AluOpType.mult,
                                )
                            # outT[h, t] = sum_s v'(s,h) e(s,t), then move the
                            # [32, T] block to its head's partition slice
                            po = psO.tile([32, T], _f32, tag="pso")
                            for sb in range(4):
                                nc.tensor.matmul(po, vprime[:, sb, :], e_nb[:, sb, :],
                                                 start=(sb == 0), stop=(sb == 3))
                            o_sb = wrkA.tile([32, T], _f32r, tag="osb")
                            nc.vector.tensor_copy(out=o_sb, in_=po)
                            nc.sync.dma_start(
                                out=attnT[nn * 32:(nn + 1) * 32, cc, b * T:(b + 1) * T],
                                in_=o_sb)

                # FF: hT = relu(W1^T @ attnT + b1)   (hT stored bf16)
                w1_sb = ap_.tile([128, ECH, HID], _f32r)
                nc.sync.dma_start(out=w1_sb, in_=w1_d[:, :].rearrange("(c p) h -> p c h", p=128).bitcast(_f32r))
                b1_sb = ap_.tile([128, HCH], _f32)
                nc.sync.dma_start(out=b1_sb, in_=b1_d[:].rearrange("(c p) -> p c", p=128))
                for hb in range(HCH):
                    for t4 in range(R // T):
                        ph = psO.tile([128, T], _f32, tag="pso")
                        for c in range(ECH):
                            nc.tensor.matmul(ph, w1_sb[:, c, hb * 128:(hb + 1) * 128],
                                             attnT[:, c, t4 * T:(t4 + 1) * T],
                                             start=(c == 0), stop=(c == ECH - 1))
                        nc.scalar.activation(out=hT[:, hb, t4 * T:(t4 + 1) * T], in_=ph,
                                             func=mybir.ActivationFunctionType.Relu,
                                             bias=b1_sb[:, hb:hb + 1], scale=1.0)

            # ---------------- fused lm_head / softmax stream ----------------
            # resident W2 shard (bf16); its pool opens after phase A's pools
            # close so it reuses their SBUF range
            with tc.tile_pool(name="w2p", bufs=1) as w2p, \
                 tc.tile_pool(name="lb", bufs=4) as lb, \
                 tc.tile_pool(name="et", bufs=2) as etp, \
                 tc.tile_pool(name="ccp", bufs=2, space="DRAM") as ccp, \
                 tc.tile_pool(name="psB", bufs=4, space="PSUM") as psB:
                w2sb = w2p.tile([128, HCH, NVT * VT], _bf16)  # 104 KB/part (tail padded)
                for vt in range(NVT):
                    w = _vt_width(vt)
                    nc.sync.dma_start(
                        out=w2sb[:, :, vt * VT:vt * VT + w],
                        in_=w2_d[:, vt * VT:vt * VT + w].rearrange("(c p) v -> p c v", p=128))
                if not b2_zero:
                    b2t = const.tile([128, VS], _f32)
                    b2_bc = bass.AP(tensor=b2_d[:].tensor, offset=0,
                                    ap=[[0, 128]] + list(b2_d[:].ap))
                    nc.sync.dma_start(out=b2t, in_=b2_bc)
                for rb in range(RB):
                    et = etp.tile([128, NVT * VT], _bf16, tag="et")
                    for vt in range(NVT):
                        w = _vt_width(vt)
                        wmm = (w + 3) // 4 * 4
                        pl = psB.tile([128, VT], _f32, tag="pl")
                        for k in range(HCH):
                            nc.tensor.matmul(pl[:, :wmm], hT[:, k, rb * 128:(rb + 1) * 128],
                                             w2sb[:, k, vt * VT:vt * VT + wmm],
                                             start=(k == 0), stop=(k == HCH - 1))
                        lt = lb.tile([128, VT], _f32, tag="lt")
                        if b2_zero:
                            # PSUM->SBUF bounce (DMA cannot read PSUM); the
                            # fused exp+row-sum reads PSUM directly in parallel
                            nc.vector.tensor_copy(out=lt[:, :w], in_=pl[:, :w])
                            nc.scalar.activation(out=et[:, vt * VT:vt * VT + w], in_=pl[:, :w],
                                                 func=mybir.ActivationFunctionType.Exp,
                                                 accum_out=S_parts[:, rb, vt:vt + 1])
                        else:
                            nc.vector.tensor_add(out=lt[:, :w], in0=pl[:, :w],
                                                 in1=b2t[:, vt * VT:vt * VT + w])
                            nc.scalar.activation(out=et[:, vt * VT:vt * VT + w], in_=lt[:, :w],
                                                 func=mybir.ActivationFunctionType.Exp,
                                                 accum_out=S_parts[:, rb, vt:vt + 1])
                        nc.sync.dma_start(
                            out=logit_o[rb * 128:(rb + 1) * 128, vt * VT:vt * VT + w],
                            in_=lt[:, :w])
                    # global row-sum for this row block (8-core AllReduce of [128,1])
                    S_loc = lb.tile([128, 1], _f32, tag="sloc")
                    nc.vector.tensor_reduce(out=S_loc, in_=S_parts[:, rb, :NVT],
                                            axis=mybir.AxisListType.X, op=mybir.AluOpType.add)
                    cc_in = ccp.tile([128, 1], _f32, tag="ccin")
                    cc_out = ccp.tile([128, 1], _f32, tag="ccout")
                    nc.sync.dma_start(out=cc_in[:], in_=S_loc)
                    nc.gpsimd.collective_compute(
                        "AllReduce", mybir.AluOpType.add,
                        replica_groups=[list(range(NCORES))],
                        ins=[cc_in.opt()], outs=[cc_out.opt()],
                    )
                    nc.sync.dma_start(out=S_glob[:, rb:rb + 1], in_=cc_out[:])
                    nc.scalar.activation(out=logS[:, rb:rb + 1], in_=S_glob[:, rb:rb + 1],
                                         func=mybir.ActivationFunctionType.Ln)
                    drec = lb.tile([128, 1], _f32, tag="drec")
                    nc.vector.reciprocal(out=drec, in_=S_glob[:, rb:rb + 1])
                    for vt in range(NVT):
                        w = _vt_width(vt)
                        ot = lb.tile([128, VT], _f32, tag="ot")
                        nc.vector.tensor_scalar(out=ot[:, :w], in0=et[:, vt * VT:vt * VT + w],
                                                scalar1=drec[:, 0:1], scalar2=None,
                                                op0=mybir.AluOpType.mult)
                        nc.sync.dma_start(
                            out=oprob_o[rb * 128:(rb + 1) * 128, vt * VT:vt * VT + w],
                            in_=ot[:, :w])

            # ---------------- loss pieces ----------------
            red1 = const.tile([128, 1], _f32)
            nc.vector.tensor_reduce(out=red1, in_=logS, axis=mybir.AxisListType.X,
                                    op=mybir.AluOpType.add)
            red2 = const.tile([1, 1], _f32)
            nc.gpsimd.tensor_reduce(out=red2, in_=red1, axis=mybir.AxisListType.C,
                                    op=mybir.AluOpType.add)
            nc.sync.dma_start(out=loss_o[0:1, 1:2], in_=red2)

            ngat = TGT_SLOTS // 128
            tgt_sb = const.tile([128, ngat], _i32)
            nc.sync.dma_start(out=tgt_sb, in_=tgt_d[:].rearrange("(g p) -> p g", p=128))
            tgw_sb = const.tile([128, ngat], _f32)
            nc.sync.dma_start(out=tgw_sb, in_=tgw_d[:].rearrange("(g p) -> p g", p=128))
            lo_ap = logit_o[:, :]
            flat_logits = bass.AP(tensor=lo_ap.tensor, offset=0, ap=[[1, R * VS], [1, 1]])
            gat = const.tile([128, ngat], _f32)
            for g in range(ngat):
                nc.gpsimd.indirect_dma_start(
                    out=gat[:, g:g + 1], out_offset=None,
                    in_=flat_logits,
                    in_offset=bass.IndirectOffsetOnAxis(ap=tgt_sb[:, g:g + 1], axis=0),
                )
            gatw = const.tile([128, ngat], _f32)
            nc.vector.tensor_mul(out=gatw, in0=gat, in1=tgw_sb)
            red3 = const.tile([128, 1], _f32)
            nc.vector.tensor_reduce(out=red3, in_=gatw, axis=mybir.AxisListType.X,
                                    op=mybir.AluOpType.add)
            red4 = const.tile([1, 1], _f32)
            nc.gpsimd.tensor_reduce(out=red4, in_=red3, axis=mybir.AxisListType.C,
                                    op=mybir.AluOpType.add)
            nc.sync.dma_start(out=loss_o[0:1, 0:1], in_=red4)

    nc.compile()
    return nc


def _get_program(b2_zero):
    key = bool(b2_zero)
    if key not in _CACHE:
        _CACHE[key] = _build_program(key)
    return _CACHE[key]


def _make_in_maps(ins):
    idx = np.ascontiguousarray(np.asarray(ins["idx"]).reshape(-1).astype(np.int32))
    targets = np.asarray(ins["targets"]).reshape(-1).astype(np.int64)
    tok_emb = np.ascontiguousarray(np.asarray(ins["tok_emb"], dtype=np.float32))
    pos_emb = np.ascontiguousarray(np.asarray(ins["pos_emb"], dtype=np.float32))
    wq = np.ascontiguousarray(np.asarray(ins["Wq"], dtype=np.float32))
    wk = np.ascontiguousarray(np.asarray(ins["Wk"], dtype=np.float32))
    wv = np.ascontiguousarray(np.asarray(ins["Wv"], dtype=np.float32))
    w1 = np.ascontiguousarray(np.asarray(ins["W1"], dtype=np.float32))
    b1 = np.ascontiguousarray(np.asarray(ins["b1"], dtype=np.float32))
    w2 = np.asarray(ins["W2"], dtype=np.float32)
    b2 = np.asarray(ins["b2"], dtype=np.float32)
    b2_zero = bool(np.all(b2 == 0.0))

    in_maps = []
    for v in range(NCORES):
        lo = v * VS
        hi = min(lo + VS, V)
        wdt = hi - lo
        w2s = np.zeros((HID, VS), dtype=ml_dtypes.bfloat16)
        w2s[:, :wdt] = w2[:, lo:hi].astype(ml_dtypes.bfloat16)
        b2s = np.zeros((VS,), dtype=np.float32)
        b2s[:wdt] = b2[lo:hi]
        rows = np.nonzero((targets >= lo) & (targets < hi))[0]
        tgt_flat = np.zeros((TGT_SLOTS,), dtype=np.int32)
        tgt_w = np.zeros((TGT_SLOTS,), dtype=np.float32)
        n = len(rows)
        tgt_flat[:n] = (rows * VS + (targets[rows] - lo)).astype(np.int32)
        tgt_w[:n] = 1.0
        in_maps.append({
            "idx_flat": idx,
            "tok_emb": tok_emb,
            "pos_emb": pos_emb,
            "Wq": wq, "Wk": wk, "Wv": wv,
            "W1": w1, "b1": b1,
            "w2s": np.ascontiguousarray(w2s),
            "b2s": b2s,
            "tgt_flat": tgt_flat,
            "tgt_w": tgt_w,
        })
    return in_maps, b2_zero


def run_sharded(ins, trace=False):
    """Build+run the SPMD program; returns (logits, loss, o_prob, results)."""
    in_maps, b2_zero = _make_in_maps(ins)
    nc = _get_program(b2_zero)
    res = run_bass_kernel_spmd(nc, in_maps, list(range(NCORES)), trace=trace)
    logits = np.concatenate(
        [res.results[v]["logits_s"] for v in range(NCORES)], axis=1)[:, :V]
    o_prob = np.concatenate(
        [res.results[v]["oprob_s"] for v in range(NCORES)], axis=1)[:, :V]
    acc = np.stack([res.results[v]["loss_acc"] for v in range(NCORES)])  # [8,1,2]
    sum_logS = float(acc[0, 0, 1])
    sum_tgt = float(acc[:, 0, 0].sum())
    loss = np.float32((sum_logS - sum_tgt) / R)
    return logits, loss, o_prob, res


def kernel(**inputs):
    logits, loss, o_prob, _ = run_sharded(inputs, trace=False)
    return logits, loss, o_prob


# revision 15
# speedup vs baseline: 1.1447x; 1.1267x over previous
"""Bass/Trainium2 kernel for nn_BigramLanguageModel (8 NeuronCores).

Strategy (vocab tensor-parallel lm_head):
  - The [B*T, vocab] logits + o_prob outputs (~824 MB) dominate: memory regime.
  - Each of the 8 cores replicates the tiny embed/attention/FF compute
    (~3 GFLOP) and owns a 1/8 shard of the vocab axis for the lm_head
    matmul, the logits/softmax, and the CE-loss pieces.
  - W2's shard lives resident in SBUF as bf16 (100.5 KB/partition), so the
    lm_head runs row-block-outer in a single fused streaming pass:
    matmul -> logits store (straight from PSUM) -> fused exp+row-sum, with
    the exp tiles kept in SBUF (bf16). Each row block's softmax denominator
    is AllReduce'd ([128] f32) across the 8 cores, pipelined behind the next
    row block's matmuls, then o_prob = e * (1/S) streams out.
  - Loss: each core gathers logit[i, tgt_i] for targets in its shard via an
    indirect DMA from its logits output and reduces on device; the host
    combines 8 partial scalars.

Self-contained: hardcodes all shapes from the problem spec.
"""

import math
import numpy as np
import ml_dtypes

import concourse.bass as bass
from concourse import bacc, mybir
from concourse.tile import TileContext
from concourse.masks import make_identity
from concourse.bass_utils import run_bass_kernel_spmd

# Problem shapes (hardcoded per contract)
V, E, BL, HID, NH = 50257, 256, 512, 1024, 8
HD = E // NH              # 32
B, T = 4, 512
R = B * T                 # 2048 rows
NCORES = 8
VS = math.ceil(V / NCORES)       # 6283 per-core vocab shard (last core padded)
VT = 512                         # vocab tile width
NVT = math.ceil(VS / VT)         # 13 tiles (last = 139)
RB = R // 128                    # 16 row blocks
ECH = E // 128                   # 2 embed chunks
HCH = HID // 128                 # 8 hidden chunks
TGT_SLOTS = 2048                 # target-gather slots (covers any distribution)

_f32 = mybir.dt.float32
_f32r = mybir.dt.float32r
_bf16 = mybir.dt.bfloat16
_i32 = mybir.dt.int32

_CACHE = {}


def _vt_width(vt):
    return VT if vt < NVT - 1 else VS - VT * (NVT - 1)


def _build_program(b2_zero):
    nc = bacc.Bacc(num_devices=NCORES)

    idx_d = nc.declare_dram_parameter("idx_flat", [R], _i32, isOutput=False)
    tok_d = nc.declare_dram_parameter("tok_emb", [V, E], _f32, isOutput=False)
    pos_d = nc.declare_dram_parameter("pos_emb", [BL, E], _f32, isOutput=False)
    wq_d = nc.declare_dram_parameter("Wq", [NH, E, HD], _f32, isOutput=False)
    wk_d = nc.declare_dram_parameter("Wk", [NH, E, HD], _f32, isOutput=False)
    wv_d = nc.declare_dram_parameter("Wv", [NH, E, HD], _f32, isOutput=False)
    w1_d = nc.declare_dram_parameter("W1", [E, HID], _f32, isOutput=False)
    b1_d = nc.declare_dram_parameter("b1", [HID], _f32, isOutput=False)
    w2_d = nc.declare_dram_parameter("w2s", [HID, VS], _bf16, isOutput=False)
    b2_d = nc.declare_dram_parameter("b2s", [VS], _f32, isOutput=False)
    tgt_d = nc.declare_dram_parameter("tgt_flat", [TGT_SLOTS], _i32, isOutput=False)
    tgw_d = nc.declare_dram_parameter("tgt_w", [TGT_SLOTS], _f32, isOutput=False)

    logit_o = nc.declare_dram_parameter("logits_s", [R, VS], _f32, isOutput=True)
    oprob_o = nc.declare_dram_parameter("oprob_s", [R, VS], _f32, isOutput=True)
    loss_o = nc.declare_dram_parameter("loss_acc", [1, 2], _f32, isOutput=True)

    inv_sqrt_c = 1.0 / math.sqrt(E)   # NOTE: reference scales scores by EMBED size

    with TileContext(nc) as tc:
        with tc.tile_pool(name="const", bufs=1) as const:
            hT = const.tile([128, HCH, R], _bf16)           # 32 KB/part
            S_parts = const.tile([128, RB, 16], _f32)
            S_glob = const.tile([128, RB], _f32)
            logS = const.tile([128, RB], _f32)

            # ---------------- Phase A: embed + attention + FF ----------------
            with tc.tile_pool(name="attn", bufs=1) as ap_, \
                 tc.tile_pool(name="wrkA", bufs=4) as wrkA, \
                 tc.tile_pool(name="psA", bufs=6, space="PSUM") as psA, \
                 tc.tile_pool(name="psO", bufs=2, space="PSUM") as psO:

                idx_sb = ap_.tile([128, RB], _i32)
                nc.sync.dma_start(out=idx_sb, in_=idx_d[:].rearrange("(c p) -> p c", p=128))

                # gather x = tok_emb[idx] (row per partition), add pos_emb
                xg = ap_.tile([128, RB, E], _f32)
                for c in range(RB):
                    nc.gpsimd.indirect_dma_start(
                        out=xg[:, c, :], out_offset=None,
                        in_=tok_d[:, :],
                        in_offset=bass.IndirectOffsetOnAxis(ap=idx_sb[:, c:c + 1], axis=0),
                    )
                pos_sb = ap_.tile([128, 4, E], _f32)
                nc.sync.dma_start(out=pos_sb, in_=pos_d[:, :].rearrange("(c p) e -> p c e", p=128))
                for c in range(RB):
                    nc.vector.tensor_add(out=xg[:, c, :], in0=xg[:, c, :], in1=pos_sb[:, c % 4, :])

                # transpose to xT [e, tokens]
                ident = ap_.tile([128, 128], _f32)
                make_identity(nc, ident)
                xT = ap_.tile([128, ECH, R], _f32r)
                for c in range(RB):
                    for e in range(ECH):
                        pt = psA.tile([128, 128], _f32, tag="ps")
                        nc.tensor.transpose(out=pt, in_=xg[:, c, e * 128:(e + 1) * 128], identity=ident)
                        nc.vector.tensor_copy(out=xT[:, e, c * 128:(c + 1) * 128], in_=pt)

                # attention weights (per-head layout [p, chunk, n, h])
                wq_sb = ap_.tile([128, ECH, NH, HD], _f32r)
                wk_sb = ap_.tile([128, ECH, NH, HD], _f32r)
                wv_sb = ap_.tile([128, ECH, NH, HD], _f32r)
                for c in range(ECH):
                    for w_d, w_sb in ((wq_d, wq_sb), (wk_d, wk_sb), (wv_d, wv_sb)):
                        nc.sync.dma_start(
                            out=w_sb[:, c],
                            in_=w_d[:, c * 128:(c + 1) * 128, :].rearrange("n p h -> p n h").bitcast(_f32r))

                # v = x @ Wv for all heads:  [tok, (n h)]
                v_all = ap_.tile([128, RB, NH * HD], _f32r)
                for tb in range(RB):
                    pv = psA.tile([128, NH * HD], _f32, tag="ps")
                    for c in range(ECH):
                        nc.tensor.matmul(
                            pv,
                            xT[:, c, tb * 128:(tb + 1) * 128],
                            wv_sb[:, c].rearrange("p n h -> p (n h)"),
                            start=(c == 0), stop=(c == ECH - 1),
                        )
                    nc.vector.tensor_copy(out=v_all[:, tb, :], in_=pv)

                attnT = ap_.tile([128, ECH, R], _f32r)
                for b in range(B):
                    for cc in range(ECH):
                        # q/k for this chunk's 4 heads in one [128, T] matmul pair:
                        # head nn lives at partitions nn*32..nn*32+32
                        pq = psA.tile([128, T], _f32, tag="ps")
                        pk = psA.tile([128, T], _f32, tag="ps")
                        for c in range(ECH):
                            nc.tensor.matmul(
                                pq, wq_sb[:, c, cc * 4:(cc + 1) * 4].rearrange("p n h -> p (n h)"),
                                xT[:, c, b * T:(b + 1) * T],
                                start=(c == 0), stop=(c == ECH - 1))
                        for c in range(ECH):
                            nc.tensor.matmul(
                                pk, wk_sb[:, c, cc * 4:(cc + 1) * 4].rearrange("p n h -> p (n h)"),
                                xT[:, c, b * T:(b + 1) * T],
                                start=(c == 0), stop=(c == ECH - 1))
                        q4 = wrkA.tile([128, T], _f32r, tag="q4")
                        k4 = wrkA.tile([128, T], _f32r, tag="k4")
                        nc.vector.tensor_copy(out=q4, in_=pq)
                        nc.vector.tensor_copy(out=k4, in_=pk)

                        for nn in range(4):
                            n = cc * 4 + nn
                            tp = {"tile_position": (nn * 32, 0)} if nn * 32 > 64 else {}
                            e_nb = wrkA.tile([128, 4, T], _f32r, tag="enb")
                            dcol = wrkA.tile([128, 4], _f32, tag="dcol")
                            for sb in range(4):
                                psc = psA.tile([128, T], _f32, tag="ps")
                                # scoresT[s, t] = k[s] . q[t]  (unscaled)
                                nc.tensor.matmul(psc,
                                                 k4[nn * 32:(nn + 1) * 32, sb * 128:(sb + 1) * 128],
                                                 q4[nn * 32:(nn + 1) * 32, :],
                                                 start=True, stop=True, **tp)
                                # exp(score / sqrt(E)); scores are tiny so no max-sub
                                nc.scalar.activation(out=e_nb[:, sb, :], in_=psc,
                                                     func=mybir.ActivationFunctionType.Exp,
                                                     scale=inv_sqrt_c)
                                # causal: keep t >= s, else 0
                                nc.gpsimd.affine_select(
                                    out=e_nb[:, sb, :], in_=e_nb[:, sb, :],
                                    compare_op=mybir.AluOpType.is_ge, fill=0.0,
                                    base=-(sb * 128), pattern=[[1, T]], channel_multiplier=-1,
                                )
                                # softmax-over-query-axis denominator, per key s
                                nc.vector.tensor_reduce(out=dcol[:, sb:sb + 1], in_=e_nb[:, sb, :],
                                                        axis=mybir.AxisListType.X,
                                                        op=mybir.AluOpType.add)
                            drec = wrkA.tile([128, 4], _f32, tag="drec")
                            nc.vector.reciprocal(out=drec, in_=dcol)
                            vprime = wrkA.tile([128, 4, HD], _f32r, tag="vp")
                            for sb in range(4):
                                nc.vector.tensor_scalar(
                                    out=vprime[:, sb, :],
                                    in0=v_all[:, b * 4 + sb, n * HD:(n + 1) * HD],
                                    scalar1=drec[:, sb:sb + 1], scalar2=None,
                                    op0=mybir.AluOpType.mult,
                                )
                            # outT[h, t] = sum_s v'(s,h) e(s,t), then move the
                            # [32, T] block to its head's partition slice
                            po = psO.tile([32, T], _f32, tag="pso")
                            for sb in range(4):
                                nc.tensor.matmul(po, vprime[:, sb, :], e_nb[:, sb, :],
                                                 start=(sb == 0), stop=(sb == 3))
                            o_sb = wrkA.tile([32, T], _f32r, tag="osb")
                            nc.vector.tensor_copy(out=o_sb, in_=po)
                            nc.sync.dma_start(
                                out=attnT[nn * 32:(nn + 1) * 32, cc, b * T:(b + 1) * T],
                                in_=o_sb)

                # FF: hT = relu(W1^T @ attnT + b1)   (hT stored bf16)
                w1_sb = ap_.tile([128, ECH, HID], _f32r)
                nc.sync.dma_start(out=w1_sb, in_=w1_d[:, :].rearrange("(c p) h -> p c h", p=128).bitcast(_f32r))
                b1_sb = ap_.tile([128, HCH], _f32)
                nc.sync.dma_start(out=b1_sb, in_=b1_d[:].rearrange("(c p) -> p c", p=128))
                for hb in range(HCH):
                    for t4 in range(R // T):
                        ph = psO.tile([128, T], _f32, tag="pso")
                        for c in range(ECH):
                            nc.tensor.matmul(ph, w1_sb[:, c, hb * 128:(hb + 1) * 128],
                                             attnT[:, c, t4 * T:(t4 + 1) * T],
                                             start=(c == 0), stop=(c == ECH - 1))
                        nc.scalar.activation(out=hT[:, hb, t4 * T:(t4 + 1) * T], in_=ph,
                                             func=mybir.ActivationFunctionType.Relu,
                                             bias=b1_sb[:, hb:hb + 1], scale=1.0)

            # ---------------- fused lm_head / softmax stream ----------------
            # resident W2 shard (bf16); its pool opens after phase A's pools
            # close so it reuses their SBUF range
            with tc.tile_pool(name="w2p", bufs=1) as w2p, \
                 tc.tile_pool(name="lb", bufs=4) as lb, \
                 tc.tile_pool(name="et", bufs=3) as etp, \
                 tc.tile_pool(name="ccp", bufs=2, space="DRAM") as ccp, \
                 tc.tile_pool(name="psB", bufs=4, space="PSUM") as psB:
                w2sb = w2p.tile([128, HCH, NVT * VT], _bf16)  # 104 KB/part (tail padded)
                for vt in range(NVT):
                    w = _vt_width(vt)
                    nc.sync.dma_start(
                        out=w2sb[:, :, vt * VT:vt * VT + w],
                        in_=w2_d[:, vt * VT:vt * VT + w].rearrange("(c p) v -> p c v", p=128))
                if not b2_zero:
                    b2t = const.tile([128, VS], _f32)
                    b2_bc = bass.AP(tensor=b2_d[:].tensor, offset=0,
                                    ap=[[0, 128]] + list(b2_d[:].ap))
                    nc.sync.dma_start(out=b2t, in_=b2_bc)
                for rb in range(RB):
                    et = etp.tile([128, NVT * VT], _bf16, tag="et")
                    for vt in range(NVT):
                        w = _vt_width(vt)
                        wmm = (w + 3) // 4 * 4
                        pl = psB.tile([128, VT], _f32, tag="pl")
                        for k in range(HCH):
                            nc.tensor.matmul(pl[:, :wmm], hT[:, k, rb * 128:(rb + 1) * 128],
                                             w2sb[:, k, vt * VT:vt * VT + wmm],
                                             start=(k == 0), stop=(k == HCH - 1))
                        lt = lb.tile([128, VT], _f32, tag="lt")
                        if b2_zero:
                            # PSUM->SBUF bounce (DMA cannot read PSUM); the
                            # fused exp+row-sum reads PSUM directly in parallel
                            nc.vector.tensor_copy(out=lt[:, :w], in_=pl[:, :w])
                            nc.scalar.activation(out=et[:, vt * VT:vt * VT + w], in_=pl[:, :w],
                                                 func=mybir.ActivationFunctionType.Exp,
                                                 accum_out=S_parts[:, rb, vt:vt + 1])
                        else:
                            nc.vector.tensor_add(out=lt[:, :w], in0=pl[:, :w],
                                                 in1=b2t[:, vt * VT:vt * VT + w])
                            nc.scalar.activation(out=et[:, vt * VT:vt * VT + w], in_=lt[:, :w],
                                                 func=mybir.ActivationFunctionType.Exp,
                                                 accum_out=S_parts[:, rb, vt:vt + 1])
                        nc.sync.dma_start(
                            out=logit_o[rb * 128:(rb + 1) * 128, vt * VT:vt * VT + w],
                            in_=lt[:, :w])
                    # global row-sum for this row block (8-core AllReduce of [128,1])
                    S_loc = lb.tile([128, 1], _f32, tag="sloc")
                    nc.vector.tensor_reduce(out=S_loc, in_=S_parts[:, rb, :NVT],
                                            axis=mybir.AxisListType.X, op=mybir.AluOpType.add)
                    cc_in = ccp.tile([128, 1], _f32, tag="ccin")
                    cc_out = ccp.tile([128, 1], _f32, tag="ccout")
                    nc.sync.dma_start(out=cc_in[:], in_=S_loc)
                    nc.gpsimd.collective_compute(
                        "AllReduce", mybir.AluOpType.add,
                        replica_groups=[list(range(NCORES))],
                        ins=[cc_in.opt()], outs=[cc_out.opt()],
                    )
                    nc.sync.dma_start(out=S_glob[:, rb:rb + 1], in_=cc_out[:])
                    nc.scalar.activation(out=logS[:, rb:rb + 1], in_=S_glob[:, rb:rb + 1],
                                         func=mybir.ActivationFunctionType.Ln)
                    drec = lb.tile([128, 1], _f32, tag="drec")
                    nc.vector.reciprocal(out=drec, in_=S_glob[:, rb:rb + 1])
                    for vt in range(NVT):
                        w = _vt_width(vt)
                        ot = lb.tile([128, VT], _f32, tag="ot")
                        nc.vector.tensor_scalar(out=ot[:, :w], in0=et[:, vt * VT:vt * VT + w],
                                                scalar1=drec[:, 0:1], scalar2=None,
                                                op0=mybir.AluOpType.mult)
                        nc.sync.dma_start(
                            out=oprob_o[rb * 128:(rb + 1) * 128, vt * VT:vt * VT + w],
                            in_=ot[:, :w])

            # ---------------- loss pieces ----------------
            red1 = const.tile([128, 1], _f32)
            nc.vector.tensor_reduce(out=red1, in_=logS, axis=mybir.AxisListType.X,
                                    op=mybir.AluOpType.add)
            red2 = const.tile([1, 1], _f32)
            nc.gpsimd.tensor_reduce(out=red2, in_=red1, axis=mybir.AxisListType.C,
                                    op=mybir.AluOpType.add)
            nc.sync.dma_start(out=loss_o[0:1, 1:2], in_=red2)

            ngat = TGT_SLOTS // 128
            tgt_sb = const.tile([128, ngat], _i32)
            nc.sync.dma_start(out=tgt_sb, in_=tgt_d[:].rearrange("(g p) -> p g", p=128))
            tgw_sb = const.tile([128, ngat], _f32)
            nc.sync.dma_start(out=tgw_sb, in_=tgw_d[:].rearrange("(g p) -> p g", p=128))
            lo_ap = logit_o[:, :]
            flat_logits = bass.AP(tensor=lo_ap.tensor, offset=0, ap=[[1, R * VS], [1, 1]])
            gat = const.tile([128, ngat], _f32)
            for g in range(ngat):
                nc.gpsimd.indirect_dma_start(
                    out=gat[:, g:g + 1], out_offset=None,
                    in_=flat_logits,
                    in_offset=bass.IndirectOffsetOnAxis(ap=tgt_sb[:, g:g + 1], axis=0),
                )
            gatw = const.tile([128, ngat], _f32)
            nc.vector.tensor_mul(out=gatw, in0=gat, in1=tgw_sb)
            red3 = const.tile([128, 1], _f32)
            nc.vector.tensor_reduce(out=red3, in_=gatw, axis=mybir.AxisListType.X,
                                    op=mybir.AluOpType.add)
            red4 = const.tile([1, 1], _f32)
            nc.gpsimd.tensor_reduce(out=red4, in_=red3, axis=mybir.AxisListType.C,
                                    op=mybir.AluOpType.add)
            nc.sync.dma_start(out=loss_o[0:1, 0:1], in_=red4)

    nc.compile()
    return nc


def _get_program(b2_zero):
    key = bool(b2_zero)
    if key not in _CACHE:
        _CACHE[key] = _build_program(key)
    return _CACHE[key]


def _make_in_maps(ins):
    idx = np.ascontiguousarray(np.asarray(ins["idx"]).reshape(-1).astype(np.int32))
    targets = np.asarray(ins["targets"]).reshape(-1).astype(np.int64)
    tok_emb = np.ascontiguousarray(np.asarray(ins["tok_emb"], dtype=np.float32))
    pos_emb = np.ascontiguousarray(np.asarray(ins["pos_emb"], dtype=np.float32))
    wq = np.ascontiguousarray(np.asarray(ins["Wq"], dtype=np.float32))
    wk = np.ascontiguousarray(np.asarray(ins["Wk"], dtype=np.float32))
    wv = np.ascontiguousarray(np.asarray(ins["Wv"], dtype=np.float32))
    w1 = np.ascontiguousarray(np.asarray(ins["W1"], dtype=np.float32))
    b1 = np.ascontiguousarray(np.asarray(ins["b1"], dtype=np.float32))
    w2 = np.asarray(ins["W2"], dtype=np.float32)
    b2 = np.asarray(ins["b2"], dtype=np.float32)
    b2_zero = bool(np.all(b2 == 0.0))

    in_maps = []
    for v in range(NCORES):
        lo = v * VS
        hi = min(lo + VS, V)
        wdt = hi - lo
        w2s = np.zeros((HID, VS), dtype=ml_dtypes.bfloat16)
        w2s[:, :wdt] = w2[:, lo:hi].astype(ml_dtypes.bfloat16)
        b2s = np.zeros((VS,), dtype=np.float32)
        b2s[:wdt] = b2[lo:hi]
        rows = np.nonzero((targets >= lo) & (targets < hi))[0]
        tgt_flat = np.zeros((TGT_SLOTS,), dtype=np.int32)
        tgt_w = np.zeros((TGT_SLOTS,), dtype=np.float32)
        n = len(rows)
        tgt_flat[:n] = (rows * VS + (targets[rows] - lo)).astype(np.int32)
        tgt_w[:n] = 1.0
        in_maps.append({
            "idx_flat": idx,
            "tok_emb": tok_emb,
            "pos_emb": pos_emb,
            "Wq": wq, "Wk": wk, "Wv": wv,
            "W1": w1, "b1": b1,
            "w2s": np.ascontiguousarray(w2s),
            "b2s": b2s,
            "tgt_flat": tgt_flat,
            "tgt_w": tgt_w,
        })
    return in_maps, b2_zero


def run_sharded(ins, trace=False):
    """Build+run the SPMD program; returns (logits, loss, o_prob, results)."""
    in_maps, b2_zero = _make_in_maps(ins)
    nc = _get_program(b2_zero)
    res = run_bass_kernel_spmd(nc, in_maps, list(range(NCORES)), trace=trace)
    logits = np.concatenate(
        [res.results[v]["logits_s"] for v in range(NCORES)], axis=1)[:, :V]
    o_prob = np.concatenate(
        [res.results[v]["oprob_s"] for v in range(NCORES)], axis=1)[:, :V]
    acc = np.stack([res.results[v]["loss_acc"] for v in range(NCORES)])  # [8,1,2]
    sum_logS = float(acc[0, 0, 1])
    sum_tgt = float(acc[:, 0, 0].sum())
    loss = np.float32((sum_logS - sum_tgt) / R)
    return logits, loss, o_prob, res


def kernel(**inputs):
    logits, loss, o_prob, _ = run_sharded(inputs, trace=False)
    return logits, loss, o_prob


# revision 16
# speedup vs baseline: 1.3017x; 1.1372x over previous
"""Bass/Trainium2 kernel for nn_BigramLanguageModel (8 NeuronCores).

Strategy (vocab tensor-parallel lm_head):
  - The [B*T, vocab] logits + o_prob outputs (~824 MB) dominate: memory regime.
  - Each of the 8 cores replicates the tiny embed/attention/FF compute
    (~3 GFLOP) and owns a 1/8 shard of the vocab axis for the lm_head
    matmul, the logits/softmax, and the CE-loss pieces.
  - W2's shard lives resident in SBUF as bf16 (100.5 KB/partition), so the
    lm_head runs row-block-outer in a single fused streaming pass:
    matmul -> logits store (straight from PSUM) -> fused exp+row-sum, with
    the exp tiles kept in SBUF (bf16). Each row block's softmax denominator
    is AllReduce'd ([128] f32) across the 8 cores, pipelined behind the next
    row block's matmuls, then o_prob = e * (1/S) streams out.
  - Loss: each core gathers logit[i, tgt_i] for targets in its shard via an
    indirect DMA from its logits output and reduces on device; the host
    combines 8 partial scalars.

Self-contained: hardcodes all shapes from the problem spec.
"""

import math
import numpy as np
import ml_dtypes

import concourse.bass as bass
from concourse import bacc, mybir
from concourse.tile import TileContext
from concourse.masks import make_identity
from concourse.bass_utils import run_bass_kernel_spmd

# Problem shapes (hardcoded per contract)
V, E, BL, HID, NH = 50257, 256, 512, 1024, 8
HD = E // NH              # 32
B, T = 4, 512
R = B * T                 # 2048 rows
NCORES = 8
VS = math.ceil(V / NCORES)       # 6283 per-core vocab shard (last core padded)
VT = 512                         # vocab tile width
NVT = math.ceil(VS / VT)         # 13 tiles (last = 139)
RB = R // 128                    # 16 row blocks
ECH = E // 128                   # 2 embed chunks
HCH = HID // 128                 # 8 hidden chunks
TGT_SLOTS = 2048                 # target-gather slots (covers any distribution)

_f32 = mybir.dt.float32
_f32r = mybir.dt.float32r
_bf16 = mybir.dt.bfloat16
_i32 = mybir.dt.int32

_CACHE = {}


def _vt_width(vt):
    return VT if vt < NVT - 1 else VS - VT * (NVT - 1)


def _build_program(b2_zero):
    nc = bacc.Bacc(num_devices=NCORES)

    idx_d = nc.declare_dram_parameter("idx_flat", [R], _i32, isOutput=False)
    tok_d = nc.declare_dram_parameter("tok_emb", [V, E], _f32, isOutput=False)
    pos_d = nc.declare_dram_parameter("pos_emb", [BL, E], _f32, isOutput=False)
    wq_d = nc.declare_dram_parameter("Wq", [NH, E, HD], _f32, isOutput=False)
    wk_d = nc.declare_dram_parameter("Wk", [NH, E, HD], _f32, isOutput=False)
    wv_d = nc.declare_dram_parameter("Wv", [NH, E, HD], _f32, isOutput=False)
    w1_d = nc.declare_dram_parameter("W1", [E, HID], _f32, isOutput=False)
    b1_d = nc.declare_dram_parameter("b1", [HID], _f32, isOutput=False)
    w2_d = nc.declare_dram_parameter("w2s", [HID, VS], _bf16, isOutput=False)
    b2_d = nc.declare_dram_parameter("b2s", [VS], _f32, isOutput=False)
    tgt_d = nc.declare_dram_parameter("tgt_flat", [TGT_SLOTS], _i32, isOutput=False)
    tgw_d = nc.declare_dram_parameter("tgt_w", [TGT_SLOTS], _f32, isOutput=False)

    logit_o = nc.declare_dram_parameter("logits_s", [R, VS], _f32, isOutput=True)
    oprob_o = nc.declare_dram_parameter("oprob_s", [R, VS], _f32, isOutput=True)
    loss_o = nc.declare_dram_parameter("loss_acc", [1, 2], _f32, isOutput=True)

    inv_sqrt_c = 1.0 / math.sqrt(E)   # NOTE: reference scales scores by EMBED size

    with TileContext(nc) as tc:
        with tc.tile_pool(name="const", bufs=1) as const:
            hT = const.tile([128, HCH, R], _bf16)           # 32 KB/part
            S_parts = const.tile([128, RB, 16], _f32)
            S_glob = const.tile([128, RB], _f32)
            logS = const.tile([128, RB], _f32)

            # ---------------- Phase A: embed + attention + FF ----------------
            with tc.tile_pool(name="attn", bufs=1) as ap_, \
                 tc.tile_pool(name="wrkA", bufs=4) as wrkA, \
                 tc.tile_pool(name="psA", bufs=6, space="PSUM") as psA, \
                 tc.tile_pool(name="psO", bufs=2, space="PSUM") as psO:

                idx_sb = ap_.tile([128, RB], _i32)
                nc.sync.dma_start(out=idx_sb, in_=idx_d[:].rearrange("(c p) -> p c", p=128))

                # gather x = tok_emb[idx] (row per partition), add pos_emb
                xg = ap_.tile([128, RB, E], _f32)
                for c in range(RB):
                    nc.gpsimd.indirect_dma_start(
                        out=xg[:, c, :], out_offset=None,
                        in_=tok_d[:, :],
                        in_offset=bass.IndirectOffsetOnAxis(ap=idx_sb[:, c:c + 1], axis=0),
                    )
                pos_sb = ap_.tile([128, 4, E], _f32)
                nc.sync.dma_start(out=pos_sb, in_=pos_d[:, :].rearrange("(c p) e -> p c e", p=128))
                for c in range(RB):
                    nc.vector.tensor_add(out=xg[:, c, :], in0=xg[:, c, :], in1=pos_sb[:, c % 4, :])

                # transpose to xT [e, tokens]
                ident = ap_.tile([128, 128], _f32)
                make_identity(nc, ident)
                xT = ap_.tile([128, ECH, R], _f32r)
                for c in range(RB):
                    for e in range(ECH):
                        pt = psA.tile([128, 128], _f32, tag="ps")
                        nc.tensor.transpose(out=pt, in_=xg[:, c, e * 128:(e + 1) * 128], identity=ident)
                        nc.vector.tensor_copy(out=xT[:, e, c * 128:(c + 1) * 128], in_=pt)

                # attention weights (per-head layout [p, chunk, n, h])
                wq_sb = ap_.tile([128, ECH, NH, HD], _f32r)
                wk_sb = ap_.tile([128, ECH, NH, HD], _f32r)
                wv_sb = ap_.tile([128, ECH, NH, HD], _f32r)
                for c in range(ECH):
                    for w_d, w_sb in ((wq_d, wq_sb), (wk_d, wk_sb), (wv_d, wv_sb)):
                        nc.sync.dma_start(
                            out=w_sb[:, c],
                            in_=w_d[:, c * 128:(c + 1) * 128, :].rearrange("n p h -> p n h").bitcast(_f32r))

                # v = x @ Wv for all heads:  [tok, (n h)]
                v_all = ap_.tile([128, RB, NH * HD], _f32r)
                for tb in range(RB):
                    pv = psA.tile([128, NH * HD], _f32, tag="ps")
                    for c in range(ECH):
                        nc.tensor.matmul(
                            pv,
                            xT[:, c, tb * 128:(tb + 1) * 128],
                            wv_sb[:, c].rearrange("p n h -> p (n h)"),
                            start=(c == 0), stop=(c == ECH - 1),
                        )
                    nc.vector.tensor_copy(out=v_all[:, tb, :], in_=pv)

                attnT = ap_.tile([128, ECH, R], _f32r)
                for b in range(B):
                    for cc in range(ECH):
                        # q/k for this chunk's 4 heads in one [128, T] matmul pair:
                        # head nn lives at partitions nn*32..nn*32+32
                        pq = psA.tile([128, T], _f32, tag="ps")
                        pk = psA.tile([128, T], _f32, tag="ps")
                        for c in range(ECH):
                            nc.tensor.matmul(
                                pq, wq_sb[:, c, cc * 4:(cc + 1) * 4].rearrange("p n h -> p (n h)"),
                                xT[:, c, b * T:(b + 1) * T],
                                start=(c == 0), stop=(c == ECH - 1))
                        for c in range(ECH):
                            nc.tensor.matmul(
                                pk, wk_sb[:, c, cc * 4:(cc + 1) * 4].rearrange("p n h -> p (n h)"),
                                xT[:, c, b * T:(b + 1) * T],
                                start=(c == 0), stop=(c == ECH - 1))
                        q4 = wrkA.tile([128, T], _f32r, tag="q4")
                        k4 = wrkA.tile([128, T], _f32r, tag="k4")
                        nc.vector.tensor_copy(out=q4, in_=pq)
                        nc.vector.tensor_copy(out=k4, in_=pk)

                        for nn in range(4):
                            n = cc * 4 + nn
                            tp = {"tile_position": (nn * 32, 0)} if nn * 32 > 64 else {}
                            e_nb = wrkA.tile([128, 4, T], _f32r, tag="enb")
                            dcol = wrkA.tile([128, 4], _f32, tag="dcol")
                            for sb in range(4):
                                psc = psA.tile([128, T], _f32, tag="ps")
                                # scoresT[s, t] = k[s] . q[t]  (unscaled)
                                nc.tensor.matmul(psc,
                                                 k4[nn * 32:(nn + 1) * 32, sb * 128:(sb + 1) * 128],
                                                 q4[nn * 32:(nn + 1) * 32, :],
                                                 start=True, stop=True, **tp)
                                # exp(score / sqrt(E)); scores are tiny so no max-sub
                                nc.scalar.activation(out=e_nb[:, sb, :], in_=psc,
                                                     func=mybir.ActivationFunctionType.Exp,
                                                     scale=inv_sqrt_c)
                                # causal: keep t >= s, else 0
                                nc.gpsimd.affine_select(
                                    out=e_nb[:, sb, :], in_=e_nb[:, sb, :],
                                    compare_op=mybir.AluOpType.is_ge, fill=0.0,
                                    base=-(sb * 128), pattern=[[1, T]], channel_multiplier=-1,
                                )
                                # softmax-over-query-axis denominator, per key s
                                nc.vector.tensor_reduce(out=dcol[:, sb:sb + 1], in_=e_nb[:, sb, :],
                                                        axis=mybir.AxisListType.X,
                                                        op=mybir.AluOpType.add)
                            drec = wrkA.tile([128, 4], _f32, tag="drec")
                            nc.vector.reciprocal(out=drec, in_=dcol)
                            vprime = wrkA.tile([128, 4, HD], _f32r, tag="vp")
                            for sb in range(4):
                                nc.vector.tensor_scalar(
                                    out=vprime[:, sb, :],
                                    in0=v_all[:, b * 4 + sb, n * HD:(n + 1) * HD],
                                    scalar1=drec[:, sb:sb + 1], scalar2=None,
                                    op0=mybir.AluOpType.mult,
                                )
                            # outT[h, t] = sum_s v'(s,h) e(s,t), then move the
                            # [32, T] block to its head's partition slice
                            po = psO.tile([32, T], _f32, tag="pso")
                            for sb in range(4):
                                nc.tensor.matmul(po, vprime[:, sb, :], e_nb[:, sb, :],
                                                 start=(sb == 0), stop=(sb == 3))
                            o_sb = wrkA.tile([32, T], _f32r, tag="osb")
                            nc.vector.tensor_copy(out=o_sb, in_=po)
                            nc.sync.dma_start(
                                out=attnT[nn * 32:(nn + 1) * 32, cc, b * T:(b + 1) * T],
                                in_=o_sb)

                # FF: hT = relu(W1^T @ attnT + b1)   (hT stored bf16)
                w1_sb = ap_.tile([128, ECH, HID], _f32r)
                nc.sync.dma_start(out=w1_sb, in_=w1_d[:, :].rearrange("(c p) h -> p c h", p=128).bitcast(_f32r))
                b1_sb = ap_.tile([128, HCH], _f32)
                nc.sync.dma_start(out=b1_sb, in_=b1_d[:].rearrange("(c p) -> p c", p=128))
                for t4 in range(R // T):
                    for hb in range(HCH):
                        ph = psO.tile([128, T], _f32, tag="pso")
                        for c in range(ECH):
                            nc.tensor.matmul(ph, w1_sb[:, c, hb * 128:(hb + 1) * 128],
                                             attnT[:, c, t4 * T:(t4 + 1) * T],
                                             start=(c == 0), stop=(c == ECH - 1))
                        nc.scalar.activation(out=hT[:, hb, t4 * T:(t4 + 1) * T], in_=ph,
                                             func=mybir.ActivationFunctionType.Relu,
                                             bias=b1_sb[:, hb:hb + 1], scale=1.0)

            # ---------------- fused lm_head / softmax stream ----------------
            # resident W2 shard (bf16); its pool opens after phase A's pools
            # close so it reuses their SBUF range
            with tc.tile_pool(name="w2p", bufs=1) as w2p, \
                 tc.tile_pool(name="lb", bufs=4) as lb, \
                 tc.tile_pool(name="et", bufs=4) as etp, \
                 tc.tile_pool(name="ccp", bufs=2, space="DRAM") as ccp, \
                 tc.tile_pool(name="psB", bufs=4, space="PSUM") as psB:
                w2sb = w2p.tile([128, HCH, NVT * VT], _bf16)  # 104 KB/part (tail padded)
                for vt in range(NVT):
                    w = _vt_width(vt)
                    nc.sync.dma_start(
                        out=w2sb[:, :, vt * VT:vt * VT + w],
                        in_=w2_d[:, vt * VT:vt * VT + w].rearrange("(c p) v -> p c v", p=128))
                if not b2_zero:
                    b2t = const.tile([128, VS], _f32)
                    b2_bc = bass.AP(tensor=b2_d[:].tensor, offset=0,
                                    ap=[[0, 128]] + list(b2_d[:].ap))
                    nc.sync.dma_start(out=b2t, in_=b2_bc)
                for grp in range(RB // 2):
                    ets = []
                    S_loc = lb.tile([128, 2], _f32, tag="sloc")
                    for half in range(2):
                        rb = grp * 2 + half
                        et = etp.tile([128, NVT * VT], _bf16, tag="et")
                        ets.append(et)
                        for vt in range(NVT):
                            w = _vt_width(vt)
                            wmm = (w + 3) // 4 * 4
                            pl = psB.tile([128, VT], _f32, tag="pl")
                            for k in range(HCH):
                                nc.tensor.matmul(pl[:, :wmm], hT[:, k, rb * 128:(rb + 1) * 128],
                                                 w2sb[:, k, vt * VT:vt * VT + wmm],
                                                 start=(k == 0), stop=(k == HCH - 1))
                            lt = lb.tile([128, VT], _f32, tag="lt")
                            if b2_zero:
                                # PSUM->SBUF bounce (DMA cannot read PSUM); the
                                # fused exp+row-sum reads PSUM directly in parallel
                                nc.vector.tensor_copy(out=lt[:, :w], in_=pl[:, :w])
                                nc.scalar.activation(out=et[:, vt * VT:vt * VT + w], in_=pl[:, :w],
                                                     func=mybir.ActivationFunctionType.Exp,
                                                     accum_out=S_parts[:, rb, vt:vt + 1])
                            else:
                                nc.vector.tensor_add(out=lt[:, :w], in0=pl[:, :w],
                                                     in1=b2t[:, vt * VT:vt * VT + w])
                                nc.scalar.activation(out=et[:, vt * VT:vt * VT + w], in_=lt[:, :w],
                                                     func=mybir.ActivationFunctionType.Exp,
                                                     accum_out=S_parts[:, rb, vt:vt + 1])
                            nc.sync.dma_start(
                                out=logit_o[rb * 128:(rb + 1) * 128, vt * VT:vt * VT + w],
                                in_=lt[:, :w])
                        nc.vector.tensor_reduce(out=S_loc[:, half:half + 1], in_=S_parts[:, rb, :NVT],
                                                axis=mybir.AxisListType.X, op=mybir.AluOpType.add)
                    # global row-sums for this pair of row blocks (8-core AllReduce)
                    cc_in = ccp.tile([128, 2], _f32, tag="ccin")
                    cc_out = ccp.tile([128, 2], _f32, tag="ccout")
                    nc.sync.dma_start(out=cc_in[:], in_=S_loc)
                    nc.gpsimd.collective_compute(
                        "AllReduce", mybir.AluOpType.add,
                        replica_groups=[list(range(NCORES))],
                        ins=[cc_in.opt()], outs=[cc_out.opt()],
                    )
                    nc.sync.dma_start(out=S_glob[:, grp * 2:grp * 2 + 2], in_=cc_out[:])
                    nc.scalar.activation(out=logS[:, grp * 2:grp * 2 + 2], in_=S_glob[:, grp * 2:grp * 2 + 2],
                                         func=mybir.ActivationFunctionType.Ln)
                    drec = lb.tile([128, 2], _f32, tag="drec")
                    nc.vector.reciprocal(out=drec, in_=S_glob[:, grp * 2:grp * 2 + 2])
                    for half in range(2):
                        rb = grp * 2 + half
                        et = ets[half]
                        for vt in range(NVT):
                            w = _vt_width(vt)
                            ot = lb.tile([128, VT], _f32, tag="ot")
                            nc.vector.tensor_scalar(out=ot[:, :w], in0=et[:, vt * VT:vt * VT + w],
                                                    scalar1=drec[:, half:half + 1], scalar2=None,
                                                    op0=mybir.AluOpType.mult)
                            nc.sync.dma_start(
                                out=oprob_o[rb * 128:(rb + 1) * 128, vt * VT:vt * VT + w],
                                in_=ot[:, :w])

            # ---------------- loss pieces ----------------
            red1 = const.tile([128, 1], _f32)
            nc.vector.tensor_reduce(out=red1, in_=logS, axis=mybir.AxisListType.X,
                                    op=mybir.AluOpType.add)
            red2 = const.tile([1, 1], _f32)
            nc.gpsimd.tensor_reduce(out=red2, in_=red1, axis=mybir.AxisListType.C,
                                    op=mybir.AluOpType.add)
            nc.sync.dma_start(out=loss_o[0:1, 1:2], in_=red2)

            ngat = TGT_SLOTS // 128
            tgt_sb = const.tile([128, ngat], _i32)
            nc.sync.dma_start(out=tgt_sb, in_=tgt_d[:].rearrange("(g p) -> p g", p=128))
            tgw_sb = const.tile([128, ngat], _f32)
            nc.sync.dma_start(out=tgw_sb, in_=tgw_d[:].rearrange("(g p) -> p g", p=128))
            lo_ap = logit_o[:, :]
            flat_logits = bass.AP(tensor=lo_ap.tensor, offset=0, ap=[[1, R * VS], [1, 1]])
            gat = const.tile([128, ngat], _f32)
            for g in range(ngat):
                nc.gpsimd.indirect_dma_start(
                    out=gat[:, g:g + 1], out_offset=None,
                    in_=flat_logits,
                    in_offset=bass.IndirectOffsetOnAxis(ap=tgt_sb[:, g:g + 1], axis=0),
                )
            gatw = const.tile([128, ngat], _f32)
            nc.vector.tensor_mul(out=gatw, in0=gat, in1=tgw_sb)
            red3 = const.tile([128, 1], _f32)
            nc.vector.tensor_reduce(out=red3, in_=gatw, axis=mybir.AxisListType.X,
                                    op=mybir.AluOpType.add)
            red4 = const.tile([1, 1], _f32)
            nc.gpsimd.tensor_reduce(out=red4, in_=red3, axis=mybir.AxisListType.C,
                                    op=mybir.AluOpType.add)
            nc.sync.dma_start(out=loss_o[0:1, 0:1], in_=red4)

    nc.compile()
    return nc


def _get_program(b2_zero):
    key = bool(b2_zero)
    if key not in _CACHE:
        _CACHE[key] = _build_program(key)
    return _CACHE[key]


def _make_in_maps(ins):
    idx = np.ascontiguousarray(np.asarray(ins["idx"]).reshape(-1).astype(np.int32))
    targets = np.asarray(ins["targets"]).reshape(-1).astype(np.int64)
    tok_emb = np.ascontiguousarray(np.asarray(ins["tok_emb"], dtype=np.float32))
    pos_emb = np.ascontiguousarray(np.asarray(ins["pos_emb"], dtype=np.float32))
    wq = np.ascontiguousarray(np.asarray(ins["Wq"], dtype=np.float32))
    wk = np.ascontiguousarray(np.asarray(ins["Wk"], dtype=np.float32))
    wv = np.ascontiguousarray(np.asarray(ins["Wv"], dtype=np.float32))
    w1 = np.ascontiguousarray(np.asarray(ins["W1"], dtype=np.float32))
    b1 = np.ascontiguousarray(np.asarray(ins["b1"], dtype=np.float32))
    w2 = np.asarray(ins["W2"], dtype=np.float32)
    b2 = np.asarray(ins["b2"], dtype=np.float32)
    b2_zero = bool(np.all(b2 == 0.0))

    in_maps = []
    for v in range(NCORES):
        lo = v * VS
        hi = min(lo + VS, V)
        wdt = hi - lo
        w2s = np.zeros((HID, VS), dtype=ml_dtypes.bfloat16)
        w2s[:, :wdt] = w2[:, lo:hi].astype(ml_dtypes.bfloat16)
        b2s = np.zeros((VS,), dtype=np.float32)
        b2s[:wdt] = b2[lo:hi]
        rows = np.nonzero((targets >= lo) & (targets < hi))[0]
        tgt_flat = np.zeros((TGT_SLOTS,), dtype=np.int32)
        tgt_w = np.zeros((TGT_SLOTS,), dtype=np.float32)
        n = len(rows)
        tgt_flat[:n] = (rows * VS + (targets[rows] - lo)).astype(np.int32)
        tgt_w[:n] = 1.0
        in_maps.append({
            "idx_flat": idx,
            "tok_emb": tok_emb,
            "pos_emb": pos_emb,
            "Wq": wq, "Wk": wk, "Wv": wv,
            "W1": w1, "b1": b1,
            "w2s": np.ascontiguousarray(w2s),
            "b2s": b2s,
            "tgt_flat": tgt_flat,
            "tgt_w": tgt_w,
        })
    return in_maps, b2_zero


def run_sharded(ins, trace=False):
    """Build+run the SPMD program; returns (logits, loss, o_prob, results)."""
    in_maps, b2_zero = _make_in_maps(ins)
    nc = _get_program(b2_zero)
    res = run_bass_kernel_spmd(nc, in_maps, list(range(NCORES)), trace=trace)
    logits = np.concatenate(
        [res.results[v]["logits_s"] for v in range(NCORES)], axis=1)[:, :V]
    o_prob = np.concatenate(
        [res.results[v]["oprob_s"] for v in range(NCORES)], axis=1)[:, :V]
    acc = np.stack([res.results[v]["loss_acc"] for v in range(NCORES)])  # [8,1,2]
    sum_logS = float(acc[0, 0, 1])
    sum_tgt = float(acc[:, 0, 0].sum())
    loss = np.float32((sum_logS - sum_tgt) / R)
    return logits, loss, o_prob, res


def kernel(**inputs):
    logits, loss, o_prob, _ = run_sharded(inputs, trace=False)
    return logits, loss, o_prob
